# revision 12
# baseline (speedup 1.0000x reference)
"""Trainium2 Bass kernel for nn_DeterministicEgnnPolicy (EGNN message passing).

Strategy (per sharding hint): shard the 1024 independent 32-node graphs
across 8 NeuronCores (128 graphs/core). On each core the fully-connected
edge structure is computed densely as all-pairs 32x32 blocks:

- "feature-major" edge tensors [128 = 2 graph-halves x 64 features,
  (i, j)] drive the edge-MLP matmuls with block-diagonal weights
  (fp16, full 128-partition contraction, fp32 PSUM accumulate).
- per-edge scalars (radial, 1/(1+sqrt(r)), u = t*s, aggregations) live in a
  "matrix" layout [128 = (gm, i), 32x32 = (gb, j)], 64x cheaper for
  DVE/ACT; batched SBUF->SBUF DMAs convert between the layouts.
- the edge phase is software-pipelined across units so the PE / ACT / DVE
  queues interleave stages of different units (no head-of-line blocking).

Graph indexing on a core: g = gb*4 + gm, gb in [0,32), gm in [0,4).
half = gb//16 (feature partitions 64*half..64*half+63).
node free index (per half): n' = gb_l*128 + gm*32 + i, gb_l = gb%16.
global node: n = gb*128 + gm*32 + i.
"""

import numpy as np

N_AGENTS = 32
BATCH = 1024
H = 64
L = 4
INV = 16
DEG = float(N_AGENTS - 1)
NCORES = 8
G_CORE = BATCH // NCORES          # 128 graphs per core
NGB = G_CORE // 4                 # 32 gb blocks per core
NGBL = NGB // 2                   # 16 per half
NNODE = NGBL * 128                # 2048 node free dim (per half)
NODES_CORE = G_CORE * N_AGENTS    # 4096
NUNIT = 64                        # units per layer (one graph-pair each)

_BUILD_CACHE = {}


# ----------------------------------------------------------------------------
# Host-side packing (pure layout permutation / weight arrangement)
# ----------------------------------------------------------------------------

def _bd(w):
    """64x64 block-diagonal lhsT [128,128] from w [64,64] (or [k,64])."""
    k = w.shape[0]
    out = np.zeros((128, 128), np.float32)
    out[0:k, 0:64] = w
    out[64:64 + k, 64:128] = w
    return out


def _bd4(w):
    """All-quadrant lhsT [128,128]: w [64,64] in each 64x64 quadrant."""
    out = np.zeros((128, 128), np.float32)
    for a in (0, 64):
        for b in (0, 64):
            out[a:a + 64, b:b + 64] = w
    return out


def _bd_rep4(wcol):
    """Replicating lhsT: out[64a+f, c] = wcol[f] for all c."""
    return np.repeat(np.tile(wcol.reshape(64, 1), (2, 1)), 128, axis=1)


def _pack_weights(inp):
    """Build wpack [128, NW*128] fp16, wfix [128, 256] f32, biaspack f32."""
    tiles = []
    names = []

    def add(name, arr):
        t = np.zeros((128, 128), np.float32)
        t[:arr.shape[0], :arr.shape[1]] = arr
        tiles.append(t)
        names.append(name)

    emb = np.zeros((128, 128), np.float32)
    emb[0:INV, 0:64] = inp["emb_W"]
    emb[64:64 + INV, 64:128] = inp["emb_W"]
    add("emb", emb)

    for l in range(L):
        We1 = inp["We1"][l]          # [130, 64]
        add(f"Wi{l}", _bd4(We1[0:64]))
        add(f"Wj{l}", _bd4(We1[64:128]))
        wsc = np.zeros((34, 128), np.float32)
        wsc[0, :] = np.tile(We1[128], 2)   # radial (h0 rows)
        wsc[1, :] = np.tile(We1[129], 2)   # edge_attr
        wsc[32, :] = np.tile(We1[128], 2)  # radial (h1 rows)
        wsc[33, :] = np.tile(We1[129], 2)
        add(f"Wsc{l}", wsc)
        add(f"We2{l}", _bd4(inp["We2"][l]))
        add(f"Wc1{l}", _bd4(inp["Wc1"][l]))
        add(f"Wc2{l}", _bd_rep4(inp["Wc2"][l][:, 0]))
        add(f"Wv1{l}", _bd4(inp["Wv1"][l]))
        add(f"Wv2{l}", _bd_rep4(inp["Wv2"][l][:, 0]))
        Wn1 = inp["Wn1"][l]          # [128, 64]
        add(f"Wn1t{l}", _bd4(Wn1[0:64]))
        add(f"Wn1b{l}", _bd4(Wn1[64:128]))
        add(f"Wn2{l}", _bd4(inp["Wn2"][l]))

    wpack = np.concatenate(tiles, axis=1).astype(np.float16)

    ident = np.eye(128, dtype=np.float32)
    delta = np.zeros((128, 128), np.float32)
    for gm in range(4):
        delta[gm, gm * 32:(gm + 1) * 32] = 1.0
    wfix = np.concatenate([ident, delta], axis=1).astype(np.float32)

    bias_cols = []
    bnames = []
    for l in range(L):
        for nm in ("be1", "be2", "bc1", "bv1", "bn1", "bn2"):
            bias_cols.append(np.tile(inp[nm][l].reshape(-1), 2))
            bnames.append(f"{nm}{l}")
        for nm in ("bv2", "bc2"):
            bias_cols.append(np.full(128, float(inp[nm][l].reshape(-1)[0]), np.float32))
            bnames.append(f"{nm}{l}")
    bias_cols.append(np.tile(inp["emb_b"], 2))
    bnames.append("embb")
    biaspack = np.stack(bias_cols, axis=1).astype(np.float32)  # [128, NB]
    return wpack, wfix, biaspack


def _arrange_inputs(obs_slice):
    """Per-core obs slice [4096, 20] -> invT fp16 [128, 2048], locvel f32."""
    obs3 = obs_slice.reshape(NGB, 128, 20)          # [gb, (gm,i), col]
    invT = np.zeros((128, NNODE), np.float32)
    inv_half0 = obs3[0:NGBL, :, 0:INV]              # [16, 128, 16]
    inv_half1 = obs3[NGBL:NGB, :, 0:INV]
    invT[0:INV, :] = np.transpose(inv_half0, (2, 0, 1)).reshape(INV, NNODE)
    invT[64:64 + INV, :] = np.transpose(inv_half1, (2, 0, 1)).reshape(INV, NNODE)
    locvel = np.ascontiguousarray(
        np.transpose(obs3[:, :, INV:INV + 4], (1, 0, 2)).reshape(128, NGB * 4)
    ).astype(np.float32)
    return invT.astype(np.float16), locvel


def _unarrange_output(outP):
    """outP [128, 64] -> [4096, 2] (n = gb*128 + p)."""
    return np.ascontiguousarray(
        outP.reshape(128, NGB, 2).transpose(1, 0, 2).reshape(NODES_CORE, 2)
    )


# ----------------------------------------------------------------------------
# Device kernel builder
# ----------------------------------------------------------------------------

def build(scale0, scale1, mean0, mean1):
    import concourse.bacc as bacc
    import concourse.tile as tile
    import concourse.mybir as mybir
    from contextlib import ExitStack

    F32 = mybir.dt.float32
    F16 = mybir.dt.float16
    AT = mybir.AluOpType
    ACTF = mybir.ActivationFunctionType

    nc = bacc.Bacc("TRN2", target_bir_lowering=False, debug=False)

    invT_d = nc.dram_tensor("invT", [128, NNODE], F16, kind="ExternalInput")
    locvel_d = nc.dram_tensor("locvel", [128, NGB * 4], F32, kind="ExternalInput")
    NW = 1 + 11 * L
    wpack_d = nc.dram_tensor("wpack", [128, NW * 128], F16, kind="ExternalInput")
    wfix_d = nc.dram_tensor("wfix", [128, 256], F32, kind="ExternalInput")
    NBIAS = 8 * L + 1
    bias_d = nc.dram_tensor("biaspack", [128, NBIAS], F32, kind="ExternalInput")
    out_d = nc.dram_tensor("out", [128, NGB * 2], F32, kind="ExternalOutput")

    # weight tile indices (must match _pack_weights order)
    widx = {}
    _wi = 0
    for nm in ("emb",):
        widx[nm] = _wi
        _wi += 1
    for l in range(L):
        for nm in ("Wi", "Wj", "Wsc", "We2", "Wc1", "Wc2", "Wv1", "Wv2",
                   "Wn1t", "Wn1b", "Wn2"):
            widx[f"{nm}{l}"] = _wi
            _wi += 1
    assert _wi == NW
    bidx = {}
    _bi = 0
    for l in range(L):
        for nm in ("be1", "be2", "bc1", "bv1", "bn1", "bn2", "bv2", "bc2"):
            bidx[f"{nm}{l}"] = _bi
            _bi += 1
    bidx["embb"] = _bi

    with tile.TileContext(nc) as tc, ExitStack() as ctx:
        st = ctx.enter_context(tc.tile_pool(name="static", bufs=1))
        eA = ctx.enter_context(tc.tile_pool(name="eA", bufs=2))
        eM = ctx.enter_context(tc.tile_pool(name="eM", bufs=2))
        eC = ctx.enter_context(tc.tile_pool(name="eC", bufs=2))
        eS = ctx.enter_context(tc.tile_pool(name="eS", bufs=2))
        eR = ctx.enter_context(tc.tile_pool(name="eR", bufs=2))
        mx = ctx.enter_context(tc.tile_pool(name="mx", bufs=1))
        # PSUM: 8 banks: ps1 2x[128,1024] + ps2 1x + ps34 1x
        ps1p = ctx.enter_context(tc.tile_pool(name="ps1p", bufs=2, space="PSUM"))
        ps2p = ctx.enter_context(tc.tile_pool(name="ps2p", bufs=1, space="PSUM"))
        ps34 = ctx.enter_context(tc.tile_pool(name="ps34", bufs=1, space="PSUM"))

        # ---- static loads ----
        wsb = st.tile([128, NW * 128], F16)
        nc.sync.dma_start(wsb[:], wpack_d.ap())
        wfx = st.tile([128, 256], F32)
        nc.sync.dma_start(wfx[:], wfix_d.ap())
        bsb = st.tile([128, NBIAS], F32)
        nc.sync.dma_start(bsb[:], bias_d.ap())
        invT = st.tile([128, NNODE], F16)
        nc.sync.dma_start(invT[:], invT_d.ap())
        locvel = st.tile([128, NGB * 4], F32)
        nc.sync.dma_start(locvel[:], locvel_d.ap())

        def W(name):
            return wsb[:, widx[name] * 128:(widx[name] + 1) * 128]

        def Bia(name):
            return bsb[:, bidx[name]:bidx[name] + 1]

        ident = wfx[:, 0:128]
        delta4 = wfx[0:4, 128:256]

        # ---- persistent state ----
        hA = st.tile([128, NNODE], F16)
        hB = st.tile([128, NNODE], F16)
        magg = st.tile([128, NNODE], F16)
        smat = st.tile([128, 1024], F32)
        rad = st.tile([128, 1024], F32)
        radf = st.tile([128, 1024], F16)
        eaf = st.tile([128, 1024], F16)
        dx = st.tile([128, 1024], F32)
        dy = st.tile([128, 1024], F32)
        locx = st.tile([128, NGB], F32)
        locy = st.tile([128, NGB], F32)
        velx = st.tile([128, NGB], F32)
        vely = st.tile([128, NGB], F32)
        phiA = st.tile([128, NGB], F32)
        phiB = st.tile([128, NGB], F32)
        hv1 = st.tile([128, NNODE], F16)
        phirep = st.tile([128, NNODE], F32)
        lxT = st.tile([32, 128], F32)
        lyT = st.tile([32, 128], F32)
        T4x = st.tile([4, 1024], F32)
        T4y = st.tile([4, 1024], F32)
        outP = st.tile([128, NGB * 2], F32)

        lv = locvel[:].rearrange("p (gb c) -> p gb c", c=4)
        nc.vector.tensor_copy(locx[:], lv[:, :, 0])
        nc.vector.tensor_copy(locy[:], lv[:, :, 1])
        nc.vector.tensor_copy(velx[:], lv[:, :, 2])
        nc.vector.tensor_copy(vely[:], lv[:, :, 3])

        def heat(lhsT_ap, rhs_ap, n=12):
            hp = ps34.tile([128, 512], F32, tag="s34")
            for _ in range(n):
                nc.tensor.matmul(hp[:], lhsT_ap, rhs_ap, start=True, stop=True)

        def radial_part(first):
            """Compute lxT/lyT, T4s, dx, dy, rad (+fp16 copies) from locx/locy."""
            for (lP, lT) in ((locx, lxT), (locy, lyT)):
                pst = ps34.tile([32, 128], F32, tag="s34")
                nc.tensor.transpose(pst[:], lP[:], ident)
                nc.vector.tensor_copy(lT[:], pst[:])
            for (lT, T4) in ((lxT, T4x), (lyT, T4y)):
                for gm in range(4):
                    nc.sync.dma_start(
                        T4[gm:gm + 1, :].rearrange("p (gb j) -> p gb j", j=32),
                        lT[:, gm * 32:(gm + 1) * 32])
            for (T4, lP, dT) in ((T4x, locx, dx), (T4y, locy, dy)):
                pss = ps2p.tile([128, 1024], F32, tag="s2")
                for k in range(2):
                    nc.tensor.matmul(pss[:, k * 512:(k + 1) * 512], delta4,
                                     T4[:, k * 512:(k + 1) * 512],
                                     start=True, stop=True)
                bc = lP[:].unsqueeze(2).broadcast_to([128, NGB, 32])
                nc.vector.tensor_tensor(
                    dT[:].rearrange("p (gb j) -> p gb j", j=32), bc,
                    pss[:].rearrange("p (gb j) -> p gb j", j=32), op=AT.subtract)
            t2 = mx.tile([128, 1024], F32, tag="mx_t2")
            nc.vector.tensor_tensor(rad[:], dx[:], dx[:], op=AT.mult)
            nc.vector.tensor_tensor(t2[:], dy[:], dy[:], op=AT.mult)
            nc.vector.tensor_tensor(rad[:], rad[:], t2[:], op=AT.add)
            with nc.allow_low_precision(reason="fp16 radial for edge MLP"):
                nc.vector.tensor_copy(radf[:], rad[:])
                if first:
                    nc.vector.tensor_copy(eaf[:], radf[:])

        radial_part(first=True)

        # rsc group DMAs: group g holds chunks (gb_l in {2g,2g+1}) x gmp,
        # free layout (gmp, u, i, gl, j) row-major per row.
        def rsc_dma(rsc, g):
            for hh in range(2):
                for (r, src) in ((32 * hh, radf), (32 * hh + 1, eaf)):
                    nc.sync.dma_start(
                        rsc[r:r + 1, :].rearrange(
                            "p (gmp u i gl j) -> p gmp u i gl j",
                            gmp=2, u=2, i=32, gl=2, j=32),
                        src[:, (2 * g + 16 * hh) * 32:(2 * g + 2 + 16 * hh) * 32]
                            .rearrange("p (gl j) -> p gl j", j=32))

        # ---- embedding: h0 = inv @ emb_W + emb_b ----
        heat(W("emb"), invT[:, 0:512])
        for u in range(NNODE // 1024):
            pse = ps1p.tile([128, 1024], F32, tag="s1")
            for k in range(2):
                nc.tensor.matmul(pse[:, k * 512:(k + 1) * 512], W("emb"),
                                 invT[:, u * 1024 + k * 512:u * 1024 + (k + 1) * 512],
                                 start=True, stop=True)
            with nc.allow_low_precision(reason="fp16 h"):
                nc.vector.tensor_scalar_add(hA[:, u * 1024:(u + 1) * 1024],
                                            pse[:], Bia("embb"))

        def node_phase(l, h, phiP):
            """phi_v for layer l from h -> phiP (matrix layout)."""
            for u in range(NNODE // 1024):
                sl = slice(u * 1024, (u + 1) * 1024)
                psv = ps2p.tile([128, 1024], F32, tag="s2")
                for k in range(2):
                    ksl = slice(u * 1024 + k * 512, u * 1024 + (k + 1) * 512)
                    for hh in range(2):
                        ib = 64 * hh
                        ob = 64 * (hh ^ k)
                        nc.tensor.matmul(psv[ob:ob + 64, k * 512:(k + 1) * 512],
                                         W(f"Wv1{l}")[ib:ib + 64, ob:ob + 64],
                                         h[ib:ib + 64, ksl],
                                         start=True, stop=True,
                                         tile_position=(ib, ob))
                with nc.allow_low_precision(reason="fp16 hv1"):
                    nc.scalar.activation(hv1[:, sl], psv[:], ACTF.Silu,
                                         bias=Bia(f"bv1{l}"))
                psv2 = ps2p.tile([128, 1024], F32, tag="s2")
                for k in range(2):
                    ksl = slice(u * 1024 + k * 512, u * 1024 + (k + 1) * 512)
                    for hh in range(2):
                        ib = 64 * (hh ^ k)
                        ob = 64 * hh
                        nc.tensor.matmul(psv2[ob:ob + 64, k * 512:(k + 1) * 512],
                                         W(f"Wv2{l}")[ib:ib + 64, ob:ob + 64],
                                         hv1[ib:ib + 64, ksl],
                                         start=True, stop=True,
                                         tile_position=(ib, ob))
                nc.vector.tensor_scalar_add(phirep[:, sl], psv2[:], Bia(f"bv2{l}"))
            # phiP[(gm,i), gb=hh*16+gb_l] = phirep[64*hh, gb_l*128+gm*32+i]
            for c in range(NGBL):
                pst = ps34.tile([128, 128], F32, tag="s34")
                nc.tensor.transpose(pst[:], phirep[:, c * 128:(c + 1) * 128], ident)
                nc.vector.tensor_copy(phiP[:, c:c + 1], pst[:, 0:1])
                nc.vector.tensor_copy(phiP[:, c + NGBL:c + NGBL + 1], pst[:, 64:65])

        node_phase(0, hA, phiA)

        for l in range(L):
            h = hA if l % 2 == 0 else hB
            h_next = hB if l % 2 == 0 else hA
            phiP = phiA if l % 2 == 0 else phiB
            phiN = phiB if l % 2 == 0 else phiA

            # ---- edge phase: software-pipelined over 64 units ----
            # unit t: gb_l = t//4, gmp = (t//2)%2, u = t%2, gl = (t//4)%2
            rsc_tiles = {}
            rsc_tiles[0] = eR.tile([64, 8192], F16, tag="rsc", name="rsc0")
            rsc_dma(rsc_tiles[0], 0)
            st_m1, st_mu, st_c1, st_s4 = {}, {}, {}, {}

            def stage1(t):
                gb_l, gmp, u, gl = t // 4, (t // 2) % 2, t % 2, (t // 4) % 2
                g = t // 8
                if t % 8 == 0 and g + 1 < 8:
                    rsc_tiles[g + 1] = eR.tile([64, 8192], F16, tag="rsc", name=f"rsc{g+1}")
                    rsc_dma(rsc_tiles[g + 1], g + 1)
                nb = gb_l * 128 + gmp * 64 + u * 32
                rv = rsc_tiles[g][0:64, :].rearrange(
                    "p (gmp u i gl j) -> p gmp u i gl j",
                    gmp=2, u=2, i=32, gl=2, j=32)
                ps1 = ps1p.tile([128, 1024], F32, tag="s1")
                for k in range(2):
                    ksl = slice(k * 512, (k + 1) * 512)
                    for hh in range(2):
                        ib = 64 * hh
                        ob = 64 * (hh ^ k)      # k1 chunks land half-swapped
                        hi = h[ib:ib + 64, nb + k * 16:nb + (k + 1) * 16]
                        hi_bc = hi.unsqueeze(2).broadcast_to([64, 16, 32])
                        hj = h[ib:ib + 64, nb:nb + 32]
                        hj_bc = hj.unsqueeze(1).broadcast_to([64, 16, 32])
                        ot = ps1[ob:ob + 64, ksl]
                        nc.tensor.matmul(ot, W(f"Wi{l}")[ib:ib + 64, ob:ob + 64],
                                         hi_bc, start=True, stop=False,
                                         tile_position=(ib, ob))
                        nc.tensor.matmul(ot, W(f"Wj{l}")[ib:ib + 64, ob:ob + 64],
                                         hj_bc, start=False, stop=False,
                                         tile_position=(ib, ob))
                        nc.tensor.matmul(ot, W(f"Wsc{l}")[32 * hh:32 * hh + 2,
                                                          ob:ob + 64],
                                         rv[32 * hh:32 * hh + 2, gmp, u,
                                            16 * k:16 * (k + 1), gl, :],
                                         start=False, stop=True,
                                         tile_position=(32 * hh, ob))
                m1s = eA.tile([128, 1024], F16, tag="m1s")
                with nc.allow_low_precision(reason="fp16 edge mlp"):
                    nc.scalar.activation(m1s[:], ps1[:], ACTF.Silu,
                                         bias=Bia(f"be1{l}"))
                st_m1[t] = m1s

            def stage2(t):
                nb = (t // 4) * 128 + ((t // 2) % 2) * 64 + (t % 2) * 32
                m1s = st_m1.pop(t)
                ps2 = ps2p.tile([128, 1024], F32, tag="s2")
                for k in range(2):
                    ksl = slice(k * 512, (k + 1) * 512)
                    for hh in range(2):
                        ib = 64 * (hh ^ k)      # m1s k1 chunks are swapped
                        ob = 64 * hh            # unswap
                        nc.tensor.matmul(ps2[ob:ob + 64, ksl],
                                         W(f"We2{l}")[ib:ib + 64, ob:ob + 64],
                                         m1s[ib:ib + 64, ksl],
                                         start=True, stop=True,
                                         tile_position=(ib, ob))
                m_u = eM.tile([128, 1024], F16, tag="m_u")
                with nc.allow_low_precision(reason="fp16 edge mlp"):
                    nc.scalar.activation(m_u[:], ps2[:], ACTF.Silu,
                                         bias=Bia(f"be2{l}"))
                    nc.vector.memset(m_u[:, 0:1024:33], 0.0)
                    nc.vector.tensor_reduce(
                        magg[:, nb:nb + 32],
                        m_u[:].rearrange("p (i j) -> p i j", j=32),
                        axis=mybir.AxisListType.X, op=AT.add)
                st_mu[t] = m_u

            def stage3(t):
                m_u = st_mu.pop(t)
                ps3 = ps34.tile([128, 1024], F32, tag="s34")
                for k in range(2):
                    ksl = slice(k * 512, (k + 1) * 512)
                    for hh in range(2):
                        ib = 64 * hh            # m_u is unswapped
                        ob = 64 * (hh ^ k)      # swap k1
                        nc.tensor.matmul(ps3[ob:ob + 64, ksl],
                                         W(f"Wc1{l}")[ib:ib + 64, ob:ob + 64],
                                         m_u[ib:ib + 64, ksl],
                                         start=True, stop=True,
                                         tile_position=(ib, ob))
                c1 = eC.tile([128, 1024], F16, tag="c1")
                with nc.allow_low_precision(reason="fp16 edge mlp"):
                    nc.scalar.activation(c1[:], ps3[:], ACTF.Silu,
                                         bias=Bia(f"bc1{l}"))
                st_c1[t] = c1

            def stage4(t):
                gb_l, gmp, u = t // 4, (t // 2) % 2, t % 2
                c1 = st_c1.pop(t)
                ps4 = ps34.tile([128, 1024], F32, tag="s34")
                for k in range(2):
                    ksl = slice(k * 512, (k + 1) * 512)
                    for hh in range(2):
                        ib = 64 * (hh ^ k)      # c1 k1 chunks are swapped
                        ob = 32 * hh            # s rows: 0 (h0), 32 (h1)
                        nc.tensor.matmul(ps4[ob:ob + 32, ksl],
                                         W(f"Wc2{l}")[ib:ib + 64, ob:ob + 32],
                                         c1[ib:ib + 64, ksl],
                                         start=True, stop=True,
                                         tile_position=(ib, ob))
                ssb = eS.tile([64, 1024], F32, tag="ssb")
                nc.vector.tensor_scalar_add(ssb[:], ps4[0:64, :],
                                            bsb[0:64, bidx[f"bc2{l}"]:bidx[f"bc2{l}"] + 1])
                pg = (gmp * 2 + u) * 32
                nc.sync.dma_start(
                    smat[pg:pg + 32, gb_l * 32:(gb_l + 1) * 32]
                        .rearrange("p (hb j) -> p hb j", hb=1),
                    ssb[0:1, :].rearrange("p (i j) -> p i j", j=32))
                nc.sync.dma_start(
                    smat[pg:pg + 32, (gb_l + 16) * 32:(gb_l + 17) * 32]
                        .rearrange("p (hb j) -> p hb j", hb=1),
                    ssb[32:33, :].rearrange("p (i j) -> p i j", j=32))

            for t in range(NUNIT + 3):
                if t < NUNIT:
                    stage1(t)
                if 1 <= t < NUNIT + 1:
                    stage2(t - 1)
                if 2 <= t < NUNIT + 2:
                    stage3(t - 2)
                if 3 <= t:
                    stage4(t - 3)

            # ---- h update (no diagonal correction: m_u diag zeroed) ----
            for u in range(NNODE // 1024):
                sl = slice(u * 1024, (u + 1) * 1024)
                psh = ps1p.tile([128, 1024], F32, tag="s1")
                for k in range(2):
                    ksl = slice(u * 1024 + k * 512, u * 1024 + (k + 1) * 512)
                    osl = slice(k * 512, (k + 1) * 512)
                    for hh in range(2):
                        ib = 64 * hh
                        ob = 64 * (hh ^ k)
                        nc.tensor.matmul(psh[ob:ob + 64, osl],
                                         W(f"Wn1t{l}")[ib:ib + 64, ob:ob + 64],
                                         h[ib:ib + 64, ksl],
                                         start=True, stop=False,
                                         tile_position=(ib, ob))
                        nc.tensor.matmul(psh[ob:ob + 64, osl],
                                         W(f"Wn1b{l}")[ib:ib + 64, ob:ob + 64],
                                         magg[ib:ib + 64, ksl],
                                         start=False, stop=True,
                                         tile_position=(ib, ob))
                hn1 = eA.tile([128, 1024], F16, tag="hn1")
                with nc.allow_low_precision(reason="fp16 edge mlp"):
                    nc.scalar.activation(hn1[:], psh[:], ACTF.Silu,
                                         bias=Bia(f"bn1{l}"))
                psh2 = ps2p.tile([128, 1024], F32, tag="s2")
                for k in range(2):
                    osl = slice(k * 512, (k + 1) * 512)
                    for hh in range(2):
                        ib = 64 * (hh ^ k)
                        ob = 64 * hh
                        nc.tensor.matmul(psh2[ob:ob + 64, osl],
                                         W(f"Wn2{l}")[ib:ib + 64, ob:ob + 64],
                                         hn1[ib:ib + 64, osl],
                                         start=True, stop=True,
                                         tile_position=(ib, ob))
                with nc.allow_low_precision(reason="fp16 h"):
                    nc.vector.scalar_tensor_tensor(
                        h_next[:, sl], psh2[:], Bia(f"bn2{l}"), h[:, sl],
                        op0=AT.add, op1=AT.add)

            # next layer's node/phi phase first: overlaps the mx phase below
            if l < L - 1:
                node_phase(l + 1, h_next, phiN)

            # ---- matrix phase: t, u, agg, vel/loc update; then radial(l+1) ----
            sq = mx.tile([128, 1024], F32, tag="mx_sq")
            nc.scalar.activation(sq[:], rad[:], ACTF.Sqrt)
            nc.vector.tensor_scalar_add(sq[:], sq[:], 1.0)
            tm = mx.tile([128, 1024], F32, tag="mx_tm")
            nc.vector.reciprocal_approx_fast(tm[:], sq[:])
            um = mx.tile([128, 1024], F32, tag="mx_um")
            nc.vector.tensor_tensor(um[:], tm[:], smat[:], op=AT.mult)
            for (dT, agg_out) in ((dx, "ax"), (dy, "ay")):
                w_ = mx.tile([128, 1024], F32, tag="mx_w")
                nc.vector.tensor_tensor(w_[:], um[:], dT[:], op=AT.mult)
                ag = mx.tile([128, NGB], F32, tag="mx_" + agg_out)
                nc.vector.tensor_reduce(
                    ag[:], w_[:].rearrange("p (gb j) -> p gb j", j=32),
                    axis=mybir.AxisListType.X, op=AT.add)
                vP = velx if agg_out == "ax" else vely
                tmp = mx.tile([128, NGB], F32, tag="mx_tmp")
                nc.vector.tensor_tensor(tmp[:], phiP[:], vP[:], op=AT.mult)
                nc.vector.scalar_tensor_tensor(vP[:], ag[:], 1.0 / DEG, tmp[:],
                                               op0=AT.mult, op1=AT.add)
            nc.vector.tensor_tensor(locx[:], locx[:], velx[:], op=AT.add)
            nc.vector.tensor_tensor(locy[:], locy[:], vely[:], op=AT.add)
            if l < L - 1:
                radial_part(first=False)

        # ---- output: outP interleaved (gb, c) ----
        ov = outP[:].rearrange("p (gb c) -> p gb c", c=2)
        nc.vector.tensor_scalar(ov[:, :, 0], velx[:], scale0, mean0,
                                op0=AT.mult, op1=AT.add)
        nc.vector.tensor_scalar(ov[:, :, 1], vely[:], scale1, mean1,
                                op0=AT.mult, op1=AT.add)
        nc.sync.dma_start(out_d.ap(), outP[:])

    nc.compile()
    return nc


# ----------------------------------------------------------------------------
# Entry point
# ----------------------------------------------------------------------------

def kernel(**inputs):
    import concourse.mybir  # noqa: F401  (ensure env importable)
    from concourse.bass_utils import run_bass_kernel_spmd

    inp = {k: np.asarray(v) for k, v in inputs.items()}
    obs = inp["obs"].astype(np.float32)
    scale = np.asarray(inp["scale"], np.float32)
    mean = np.asarray(inp["mean"], np.float32)

    key = (float(scale[0]), float(scale[1]), float(mean[0]), float(mean[1]))
    if key not in _BUILD_CACHE:
        _BUILD_CACHE[key] = build(*key)
    nc = _BUILD_CACHE[key]

    wpack, wfix, biaspack = _pack_weights(inp)
    in_maps = []
    for c in range(NCORES):
        invT, locvel = _arrange_inputs(obs[c * NODES_CORE:(c + 1) * NODES_CORE])
        in_maps.append({"invT": invT, "locvel": locvel, "wpack": wpack,
                        "wfix": wfix, "biaspack": biaspack})
    res = run_bass_kernel_spmd(nc, in_maps, list(range(NCORES)))
    outs = [_unarrange_output(res.results[c]["out"]) for c in range(NCORES)]
    return np.concatenate(outs, axis=0)


# revision 15
# speedup vs baseline: 1.1607x; 1.1607x over previous
"""Trainium2 Bass kernel for nn_DeterministicEgnnPolicy (EGNN message passing).

Strategy (per sharding hint): shard the 1024 independent 32-node graphs
across 8 NeuronCores (128 graphs/core). On each core the fully-connected
edge structure is computed densely as all-pairs 32x32 blocks:

- "feature-major" edge tensors [128 = 2 graph-halves x 64 features,
  (i, j)] drive the edge-MLP matmuls with block-diagonal weights
  (fp16, full 128-partition contraction, fp32 PSUM accumulate).
- per-edge scalars (radial, 1/(1+sqrt(r)), u = t*s, aggregations) live in a
  "matrix" layout [128 = (gm, i), 32x32 = (gb, j)], 64x cheaper for
  DVE/ACT; batched SBUF->SBUF DMAs convert between the layouts.
- the edge phase is software-pipelined across units so the PE / ACT / DVE
  queues interleave stages of different units (no head-of-line blocking).

Graph indexing on a core: g = gb*4 + gm, gb in [0,32), gm in [0,4).
half = gb//16 (feature partitions 64*half..64*half+63).
node free index (per half): n' = gb_l*128 + gm*32 + i, gb_l = gb%16.
global node: n = gb*128 + gm*32 + i.
"""

import numpy as np

N_AGENTS = 32
BATCH = 1024
H = 64
L = 4
INV = 16
DEG = float(N_AGENTS - 1)
NCORES = 8
G_CORE = BATCH // NCORES          # 128 graphs per core
NGB = G_CORE // 4                 # 32 gb blocks per core
NGBL = NGB // 2                   # 16 per half
NNODE = NGBL * 128                # 2048 node free dim (per half)
NODES_CORE = G_CORE * N_AGENTS    # 4096
NUNIT = 64                        # units per layer (one graph-pair each)

_BUILD_CACHE = {}


# ----------------------------------------------------------------------------
# Host-side packing (pure layout permutation / weight arrangement)
# ----------------------------------------------------------------------------

def _bd(w):
    """64x64 block-diagonal lhsT [128,128] from w [64,64] (or [k,64])."""
    k = w.shape[0]
    out = np.zeros((128, 128), np.float32)
    out[0:k, 0:64] = w
    out[64:64 + k, 64:128] = w
    return out


def _bd_rep(wcol):
    """Replicating lhsT: out[64h+f, 64h+f'] = wcol[f] for all f'."""
    out = np.zeros((128, 128), np.float32)
    col = wcol.reshape(64, 1)
    out[0:64, 0:64] = np.repeat(col, 64, axis=1)
    out[64:128, 64:128] = np.repeat(col, 64, axis=1)
    return out


def _bd4(w):
    """All-quadrant lhsT [128,128]: w [64,64] in each 64x64 quadrant."""
    out = np.zeros((128, 128), np.float32)
    for a in (0, 64):
        for b in (0, 64):
            out[a:a + 64, b:b + 64] = w
    return out


def _bd_rep4(wcol):
    """Replicating lhsT: out[64a+f, c] = wcol[f] for all c."""
    return np.repeat(np.tile(wcol.reshape(64, 1), (2, 1)), 128, axis=1)


def _pack_weights(inp):
    """Build wpack [128, NW*128] fp16, wfix [128, 256] f32, biaspack f32."""
    tiles = []
    names = []

    def add(name, arr):
        t = np.zeros((128, 128), np.float32)
        t[:arr.shape[0], :arr.shape[1]] = arr
        tiles.append(t)
        names.append(name)

    emb = np.zeros((128, 128), np.float32)
    emb[0:INV, 0:64] = inp["emb_W"]
    emb[64:64 + INV, 64:128] = inp["emb_W"]
    add("emb", emb)

    for l in range(L):
        We1 = inp["We1"][l]          # [130, 64]
        add(f"Wi{l}", _bd(We1[0:64]))
        add(f"Wj{l}", _bd(We1[64:128]))
        wsc = np.zeros((4, 128), np.float32)
        wsc[0, 0:64] = We1[128]      # radial, half0
        wsc[1, 0:64] = We1[129]      # edge_attr, half0
        wsc[2, 64:128] = We1[128]
        wsc[3, 64:128] = We1[129]
        add(f"Wsc{l}", wsc)
        add(f"We2{l}", _bd(inp["We2"][l]))
        add(f"Wc1{l}", _bd(inp["Wc1"][l]))
        add(f"Wc2{l}", _bd_rep(inp["Wc2"][l][:, 0]))
        add(f"Wv1{l}", _bd(inp["Wv1"][l]))
        add(f"Wv2{l}", _bd_rep(inp["Wv2"][l][:, 0]))
        Wn1 = inp["Wn1"][l]          # [128, 64]
        add(f"Wn1t{l}", _bd(Wn1[0:64]))
        add(f"Wn1b{l}", _bd(Wn1[64:128]))
        add(f"Wn2{l}", _bd(inp["Wn2"][l]))

    wpack = np.concatenate(tiles, axis=1).astype(np.float16)

    ident = np.eye(128, dtype=np.float32)
    delta = np.zeros((128, 128), np.float32)
    for gm in range(4):
        delta[gm, gm * 32:(gm + 1) * 32] = 1.0
    wfix = np.concatenate([ident, delta], axis=1).astype(np.float32)

    bias_cols = []
    bnames = []
    for l in range(L):
        for nm in ("be1", "be2", "bc1", "bv1", "bn1", "bn2"):
            bias_cols.append(np.tile(inp[nm][l].reshape(-1), 2))
            bnames.append(f"{nm}{l}")
        for nm in ("bv2", "bc2"):
            bias_cols.append(np.full(128, float(inp[nm][l].reshape(-1)[0]), np.float32))
            bnames.append(f"{nm}{l}")
    bias_cols.append(np.tile(inp["emb_b"], 2))
    bnames.append("embb")
    biaspack = np.stack(bias_cols, axis=1).astype(np.float32)  # [128, NB]
    return wpack, wfix, biaspack


def _arrange_inputs(obs_slice):
    """Per-core obs slice [4096, 20] -> invT fp16 [128, 2048], locvel f32."""
    obs3 = obs_slice.reshape(NGB, 128, 20)          # [gb, (gm,i), col]
    invT = np.zeros((128, NNODE), np.float32)
    inv_half0 = obs3[0:NGBL, :, 0:INV]              # [16, 128, 16]
    inv_half1 = obs3[NGBL:NGB, :, 0:INV]
    invT[0:INV, :] = np.transpose(inv_half0, (2, 0, 1)).reshape(INV, NNODE)
    invT[64:64 + INV, :] = np.transpose(inv_half1, (2, 0, 1)).reshape(INV, NNODE)
    locvel = np.ascontiguousarray(
        np.transpose(obs3[:, :, INV:INV + 4], (1, 0, 2)).reshape(128, NGB * 4)
    ).astype(np.float32)
    return invT.astype(np.float16), locvel


def _unarrange_output(outP):
    """outP [128, 64] -> [4096, 2] (n = gb*128 + p)."""
    return np.ascontiguousarray(
        outP.reshape(128, NGB, 2).transpose(1, 0, 2).reshape(NODES_CORE, 2)
    )


# ----------------------------------------------------------------------------
# Device kernel builder
# ----------------------------------------------------------------------------

def build(scale0, scale1, mean0, mean1):
    import concourse.bacc as bacc
    import concourse.tile as tile
    import concourse.mybir as mybir
    from contextlib import ExitStack

    F32 = mybir.dt.float32
    F16 = mybir.dt.float16
    AT = mybir.AluOpType
    ACTF = mybir.ActivationFunctionType

    nc = bacc.Bacc("TRN2", target_bir_lowering=False, debug=False)

    invT_d = nc.dram_tensor("invT", [128, NNODE], F16, kind="ExternalInput")
    locvel_d = nc.dram_tensor("locvel", [128, NGB * 4], F32, kind="ExternalInput")
    NW = 1 + 11 * L
    wpack_d = nc.dram_tensor("wpack", [128, NW * 128], F16, kind="ExternalInput")
    wfix_d = nc.dram_tensor("wfix", [128, 256], F32, kind="ExternalInput")
    NBIAS = 8 * L + 1
    bias_d = nc.dram_tensor("biaspack", [128, NBIAS], F32, kind="ExternalInput")
    out_d = nc.dram_tensor("out", [128, NGB * 2], F32, kind="ExternalOutput")

    # weight tile indices (must match _pack_weights order)
    widx = {}
    _wi = 0
    for nm in ("emb",):
        widx[nm] = _wi
        _wi += 1
    for l in range(L):
        for nm in ("Wi", "Wj", "Wsc", "We2", "Wc1", "Wc2", "Wv1", "Wv2",
                   "Wn1t", "Wn1b", "Wn2"):
            widx[f"{nm}{l}"] = _wi
            _wi += 1
    assert _wi == NW
    bidx = {}
    _bi = 0
    for l in range(L):
        for nm in ("be1", "be2", "bc1", "bv1", "bn1", "bn2", "bv2", "bc2"):
            bidx[f"{nm}{l}"] = _bi
            _bi += 1
    bidx["embb"] = _bi

    with tile.TileContext(nc) as tc, ExitStack() as ctx:
        st = ctx.enter_context(tc.tile_pool(name="static", bufs=1))
        eA = ctx.enter_context(tc.tile_pool(name="eA", bufs=2))
        eM = ctx.enter_context(tc.tile_pool(name="eM", bufs=2))
        eC = ctx.enter_context(tc.tile_pool(name="eC", bufs=2))
        eS = ctx.enter_context(tc.tile_pool(name="eS", bufs=2))
        eR = ctx.enter_context(tc.tile_pool(name="eR", bufs=2))
        mx = ctx.enter_context(tc.tile_pool(name="mx", bufs=1))
        # PSUM: 8 banks: ps1/ps2/ps3/ps4 each 1x[128,1024]
        ps1p = ctx.enter_context(tc.tile_pool(name="ps1p", bufs=1, space="PSUM"))
        ps2p = ctx.enter_context(tc.tile_pool(name="ps2p", bufs=1, space="PSUM"))
        ps3p = ctx.enter_context(tc.tile_pool(name="ps3p", bufs=1, space="PSUM"))
        ps4p = ctx.enter_context(tc.tile_pool(name="ps4p", bufs=1, space="PSUM"))

        # ---- static loads ----
        wsb = st.tile([128, NW * 128], F16)
        nc.sync.dma_start(wsb[:], wpack_d.ap())
        wfx = st.tile([128, 256], F32)
        nc.sync.dma_start(wfx[:], wfix_d.ap())
        bsb = st.tile([128, NBIAS], F32)
        nc.sync.dma_start(bsb[:], bias_d.ap())
        invT = st.tile([128, NNODE], F16)
        nc.sync.dma_start(invT[:], invT_d.ap())
        locvel = st.tile([128, NGB * 4], F32)
        nc.sync.dma_start(locvel[:], locvel_d.ap())

        def W(name):
            return wsb[:, widx[name] * 128:(widx[name] + 1) * 128]

        def Bia(name):
            return bsb[:, bidx[name]:bidx[name] + 1]

        ident = wfx[:, 0:128]
        delta4 = wfx[0:4, 128:256]

        # ---- persistent state ----
        hA = st.tile([128, NNODE], F16)
        hB = st.tile([128, NNODE], F16)
        magg = st.tile([128, NNODE], F16)
        smat = st.tile([128, 1024], F32)
        rad = st.tile([128, 1024], F32)
        radf = st.tile([128, 1024], F16)
        eaf = st.tile([128, 1024], F16)
        dx = st.tile([128, 1024], F32)
        dy = st.tile([128, 1024], F32)
        locx = st.tile([128, NGB], F32)
        locy = st.tile([128, NGB], F32)
        velx = st.tile([128, NGB], F32)
        vely = st.tile([128, NGB], F32)
        phiA = st.tile([128, NGB], F32)
        phiB = st.tile([128, NGB], F32)
        hv1 = st.tile([128, NNODE], F16)
        phirep = st.tile([128, NNODE], F32)
        lxT = st.tile([32, 128], F32)
        lyT = st.tile([32, 128], F32)
        T4x = st.tile([4, 1024], F32)
        T4y = st.tile([4, 1024], F32)
        outP = st.tile([128, NGB * 2], F32)

        lv = locvel[:].rearrange("p (gb c) -> p gb c", c=4)
        nc.vector.tensor_copy(locx[:], lv[:, :, 0])
        nc.vector.tensor_copy(locy[:], lv[:, :, 1])
        nc.vector.tensor_copy(velx[:], lv[:, :, 2])
        nc.vector.tensor_copy(vely[:], lv[:, :, 3])

        def heat(lhsT_ap, rhs_ap, n=12):
            hp = ps4p.tile([128, 512], F32, tag="s4")
            for _ in range(n):
                nc.tensor.matmul(hp[:], lhsT_ap, rhs_ap, start=True, stop=True)

        def radial_part(first):
            """Compute lxT/lyT, T4s, dx, dy, rad (+fp16 copies) from locx/locy."""
            for (lP, lT) in ((locx, lxT), (locy, lyT)):
                pst = ps4p.tile([32, 128], F32, tag="s4")
                nc.tensor.transpose(pst[:], lP[:], ident)
                nc.vector.tensor_copy(lT[:], pst[:])
            for (lT, T4) in ((lxT, T4x), (lyT, T4y)):
                for gm in range(4):
                    nc.sync.dma_start(
                        T4[gm:gm + 1, :].rearrange("p (gb j) -> p gb j", j=32),
                        lT[:, gm * 32:(gm + 1) * 32])
            for (T4, lP, dT) in ((T4x, locx, dx), (T4y, locy, dy)):
                pss = ps2p.tile([128, 1024], F32, tag="s2")
                for k in range(2):
                    nc.tensor.matmul(pss[:, k * 512:(k + 1) * 512], delta4,
                                     T4[:, k * 512:(k + 1) * 512],
                                     start=True, stop=True)
                bc = lP[:].unsqueeze(2).broadcast_to([128, NGB, 32])
                nc.vector.tensor_tensor(
                    dT[:].rearrange("p (gb j) -> p gb j", j=32), bc,
                    pss[:].rearrange("p (gb j) -> p gb j", j=32), op=AT.subtract)
            t2 = mx.tile([128, 1024], F32, tag="mx_t2")
            nc.vector.tensor_tensor(rad[:], dx[:], dx[:], op=AT.mult)
            nc.vector.tensor_tensor(t2[:], dy[:], dy[:], op=AT.mult)
            nc.vector.tensor_tensor(rad[:], rad[:], t2[:], op=AT.add)
            with nc.allow_low_precision(reason="fp16 radial for edge MLP"):
                nc.vector.tensor_copy(radf[:], rad[:])
                if first:
                    nc.vector.tensor_copy(eaf[:], radf[:])

        radial_part(first=True)

        # rsc group DMAs: group g holds chunks (gb_l in {2g,2g+1}) x gmp,
        # free layout (gmp, u, i, gl, j) row-major per row.
        def rsc_dma(rsc, g):
            for hh in range(2):
                for (r, src) in ((2 * hh, radf), (2 * hh + 1, eaf)):
                    nc.sync.dma_start(
                        rsc[r:r + 1, :].rearrange(
                            "p (gmp u i gl j) -> p gmp u i gl j",
                            gmp=2, u=2, i=32, gl=2, j=32),
                        src[:, (2 * g + 16 * hh) * 32:(2 * g + 2 + 16 * hh) * 32]
                            .rearrange("p (gl j) -> p gl j", j=32))

        # ---- embedding: h0 = inv @ emb_W + emb_b ----
        heat(W("emb"), invT[:, 0:512])
        for u in range(NNODE // 1024):
            pse = ps1p.tile([128, 1024], F32, tag="s1")
            for k in range(2):
                nc.tensor.matmul(pse[:, k * 512:(k + 1) * 512], W("emb"),
                                 invT[:, u * 1024 + k * 512:u * 1024 + (k + 1) * 512],
                                 start=True, stop=True)
            with nc.allow_low_precision(reason="fp16 h"):
                nc.vector.tensor_scalar_add(hA[:, u * 1024:(u + 1) * 1024],
                                            pse[:], Bia("embb"))

        def node_phase(l, h, phiP):
            """phi_v for layer l from h -> phiP (matrix layout)."""
            for u in range(NNODE // 1024):
                sl = slice(u * 1024, (u + 1) * 1024)
                psv = ps2p.tile([128, 1024], F32, tag="s2")
                for k in range(2):
                    ksl = slice(u * 1024 + k * 512, u * 1024 + (k + 1) * 512)
                    nc.tensor.matmul(psv[:, k * 512:(k + 1) * 512],
                                     W(f"Wv1{l}"), h[:, ksl],
                                     start=True, stop=True)
                with nc.allow_low_precision(reason="fp16 hv1"):
                    nc.scalar.activation(hv1[:, sl], psv[:], ACTF.Silu,
                                         bias=Bia(f"bv1{l}"))
                psv2 = ps2p.tile([128, 1024], F32, tag="s2")
                for k in range(2):
                    nc.tensor.matmul(psv2[:, k * 512:(k + 1) * 512],
                                     W(f"Wv2{l}"),
                                     hv1[:, u * 1024 + k * 512:u * 1024 + (k + 1) * 512],
                                     start=True, stop=True)
                nc.vector.tensor_scalar_add(phirep[:, sl], psv2[:], Bia(f"bv2{l}"))
            # phiP[(gm,i), gb=hh*16+gb_l] = phirep[64*hh, gb_l*128+gm*32+i]
            for c in range(NGBL):
                pst = ps4p.tile([128, 128], F32, tag="s4")
                nc.tensor.transpose(pst[:], phirep[:, c * 128:(c + 1) * 128], ident)
                nc.vector.tensor_copy(phiP[:, c:c + 1], pst[:, 0:1])
                nc.vector.tensor_copy(phiP[:, c + NGBL:c + NGBL + 1], pst[:, 64:65])

        node_phase(0, hA, phiA)

        for l in range(L):
            h = hA if l % 2 == 0 else hB
            h_next = hB if l % 2 == 0 else hA
            phiP = phiA if l % 2 == 0 else phiB
            phiN = phiB if l % 2 == 0 else phiA

            # ---- edge phase: software-pipelined over 64 units ----
            # unit t: gb_l = t//4, gmp = (t//2)%2, u = t%2, gl = (t//4)%2
            rsc_tiles = {}
            rsc_tiles[0] = eR.tile([4, 8192], F16, tag="rsc", name="rsc0")
            rsc_dma(rsc_tiles[0], 0)
            st_m1, st_mu, st_c1, st_s4 = {}, {}, {}, {}

            def stage1(t):
                gb_l, gmp, u, gl = t // 4, (t // 2) % 2, t % 2, (t // 4) % 2
                g = t // 8
                if t % 8 == 0 and g + 1 < 8:
                    rsc_tiles[g + 1] = eR.tile([4, 8192], F16, tag="rsc", name=f"rsc{g+1}")
                    rsc_dma(rsc_tiles[g + 1], g + 1)
                nb = gb_l * 128 + gmp * 64 + u * 32
                rv = rsc_tiles[g][0:4, :].rearrange(
                    "p (gmp u i gl j) -> p gmp u i gl j",
                    gmp=2, u=2, i=32, gl=2, j=32)
                ps1 = ps1p.tile([128, 1024], F32, tag="s1")
                for k in range(2):
                    ksl = slice(k * 512, (k + 1) * 512)
                    hi = h[:, nb + k * 16:nb + (k + 1) * 16]
                    hi_bc = hi.unsqueeze(2).broadcast_to([128, 16, 32])
                    hj = h[:, nb:nb + 32]
                    hj_bc = hj.unsqueeze(1).broadcast_to([128, 16, 32])
                    nc.tensor.matmul(ps1[:, ksl], W(f"Wi{l}"),
                                     hi_bc, start=True, stop=False)
                    nc.tensor.matmul(ps1[:, ksl], W(f"Wj{l}"),
                                     hj_bc, start=False, stop=False)
                    nc.tensor.matmul(ps1[:, ksl], W(f"Wsc{l}")[0:4, :],
                                     rv[:, gmp, u, 16 * k:16 * (k + 1), gl, :],
                                     start=False, stop=True)
                m1s = eA.tile([128, 1024], F16, tag="m1s")
                with nc.allow_low_precision(reason="fp16 edge mlp"):
                    nc.scalar.activation(m1s[:], ps1[:], ACTF.Silu,
                                         bias=Bia(f"be1{l}"))
                st_m1[t] = m1s

            def stage2(t):
                nb = (t // 4) * 128 + ((t // 2) % 2) * 64 + (t % 2) * 32
                m1s = st_m1.pop(t)
                ps2 = ps2p.tile([128, 1024], F32, tag="s2")
                for k in range(2):
                    ksl = slice(k * 512, (k + 1) * 512)
                    nc.tensor.matmul(ps2[:, ksl], W(f"We2{l}"),
                                     m1s[:, ksl], start=True, stop=True)
                m_u = eM.tile([128, 1024], F16, tag="m_u")
                with nc.allow_low_precision(reason="fp16 edge mlp"):
                    nc.scalar.activation(m_u[:], ps2[:], ACTF.Silu,
                                         bias=Bia(f"be2{l}"))
                    nc.vector.memset(m_u[:, 0:1024:33], 0.0)
                    nc.vector.tensor_reduce(
                        magg[:, nb:nb + 32],
                        m_u[:].rearrange("p (i j) -> p i j", j=32),
                        axis=mybir.AxisListType.X, op=AT.add)
                st_mu[t] = m_u

            def stage3(t):
                m_u = st_mu.pop(t)
                ps3 = ps3p.tile([128, 1024], F32, tag="s3")
                for k in range(2):
                    ksl = slice(k * 512, (k + 1) * 512)
                    nc.tensor.matmul(ps3[:, ksl], W(f"Wc1{l}"),
                                     m_u[:, ksl], start=True, stop=True)
                c1 = eC.tile([128, 1024], F16, tag="c1")
                with nc.allow_low_precision(reason="fp16 edge mlp"):
                    nc.scalar.activation(c1[:], ps3[:], ACTF.Silu,
                                         bias=Bia(f"bc1{l}"))
                st_c1[t] = c1

            def stage4(t):
                gb_l, gmp, u = t // 4, (t // 2) % 2, t % 2
                c1 = st_c1.pop(t)
                ps4 = ps4p.tile([128, 1024], F32, tag="s4")
                for k in range(2):
                    ksl = slice(k * 512, (k + 1) * 512)
                    nc.tensor.matmul(ps4[:, ksl], W(f"Wc2{l}"),
                                     c1[:, ksl], start=True, stop=True)
                ssb = eS.tile([128, 1024], F32, tag="ssb")
                nc.vector.tensor_scalar_add(ssb[:], ps4[:], Bia(f"bc2{l}"))
                pg = (gmp * 2 + u) * 32
                nc.sync.dma_start(
                    smat[pg:pg + 32, gb_l * 32:(gb_l + 1) * 32]
                        .rearrange("p (hb j) -> p hb j", hb=1),
                    ssb[0:1, :].rearrange("p (i j) -> p i j", j=32))
                nc.sync.dma_start(
                    smat[pg:pg + 32, (gb_l + 16) * 32:(gb_l + 17) * 32]
                        .rearrange("p (hb j) -> p hb j", hb=1),
                    ssb[64:65, :].rearrange("p (i j) -> p i j", j=32))

            # ---- h update (no diagonal correction: m_u diag zeroed) ----
            def h_update(u):
                sl = slice(u * 1024, (u + 1) * 1024)
                psh = ps1p.tile([128, 1024], F32, tag="s1")
                for k in range(2):
                    ksl = slice(u * 1024 + k * 512, u * 1024 + (k + 1) * 512)
                    osl = slice(k * 512, (k + 1) * 512)
                    nc.tensor.matmul(psh[:, osl], W(f"Wn1t{l}"),
                                     h[:, ksl], start=True, stop=False)
                    nc.tensor.matmul(psh[:, osl], W(f"Wn1b{l}"),
                                     magg[:, ksl], start=False, stop=True)
                hn1 = eA.tile([128, 1024], F16, tag="hn1")
                with nc.allow_low_precision(reason="fp16 edge mlp"):
                    nc.scalar.activation(hn1[:], psh[:], ACTF.Silu,
                                         bias=Bia(f"bn1{l}"))
                psh2 = ps2p.tile([128, 1024], F32, tag="s2")
                for k in range(2):
                    osl = slice(k * 512, (k + 1) * 512)
                    nc.tensor.matmul(psh2[:, osl], W(f"Wn2{l}"),
                                     hn1[:, osl], start=True, stop=True)
                with nc.allow_low_precision(reason="fp16 h"):
                    nc.vector.scalar_tensor_tensor(
                        h_next[:, sl], psh2[:], Bia(f"bn2{l}"), h[:, sl],
                        op0=AT.add, op1=AT.add)


            for t in range(NUNIT + 3):
                if t < NUNIT:
                    stage1(t)
                if 1 <= t < NUNIT + 1:
                    stage2(t - 1)
                if 2 <= t < NUNIT + 2:
                    stage3(t - 2)
                if 3 <= t:
                    stage4(t - 3)
                if t == 38:
                    h_update(0)

            h_update(1)

            # next layer's node/phi phase first: overlaps the mx phase below
            if l < L - 1:
                node_phase(l + 1, h_next, phiN)

            # ---- matrix phase: t, u, agg, vel/loc update; then radial(l+1) ----
            sq = mx.tile([128, 1024], F32, tag="mx_sq")
            nc.scalar.activation(sq[:], rad[:], ACTF.Sqrt)
            nc.vector.tensor_scalar_add(sq[:], sq[:], 1.0)
            tm = mx.tile([128, 1024], F32, tag="mx_tm")
            nc.vector.reciprocal_approx_fast(tm[:], sq[:])
            um = mx.tile([128, 1024], F32, tag="mx_um")
            nc.vector.tensor_tensor(um[:], tm[:], smat[:], op=AT.mult)
            for (dT, agg_out) in ((dx, "ax"), (dy, "ay")):
                w_ = mx.tile([128, 1024], F32, tag="mx_w")
                nc.vector.tensor_tensor(w_[:], um[:], dT[:], op=AT.mult)
                ag = mx.tile([128, NGB], F32, tag="mx_" + agg_out)
                nc.vector.tensor_reduce(
                    ag[:], w_[:].rearrange("p (gb j) -> p gb j", j=32),
                    axis=mybir.AxisListType.X, op=AT.add)
                vP = velx if agg_out == "ax" else vely
                tmp = mx.tile([128, NGB], F32, tag="mx_tmp")
                nc.vector.tensor_tensor(tmp[:], phiP[:], vP[:], op=AT.mult)
                nc.vector.scalar_tensor_tensor(vP[:], ag[:], 1.0 / DEG, tmp[:],
                                               op0=AT.mult, op1=AT.add)
            nc.vector.tensor_tensor(locx[:], locx[:], velx[:], op=AT.add)
            nc.vector.tensor_tensor(locy[:], locy[:], vely[:], op=AT.add)
            if l < L - 1:
                radial_part(first=False)

        # ---- output: outP interleaved (gb, c) ----
        ov = outP[:].rearrange("p (gb c) -> p gb c", c=2)
        nc.vector.tensor_scalar(ov[:, :, 0], velx[:], scale0, mean0,
                                op0=AT.mult, op1=AT.add)
        nc.vector.tensor_scalar(ov[:, :, 1], vely[:], scale1, mean1,
                                op0=AT.mult, op1=AT.add)
        nc.sync.dma_start(out_d.ap(), outP[:])

    nc.compile()
    return nc


# ----------------------------------------------------------------------------
# Entry point
# ----------------------------------------------------------------------------

def kernel(**inputs):
    import concourse.mybir  # noqa: F401  (ensure env importable)
    from concourse.bass_utils import run_bass_kernel_spmd

    inp = {k: np.asarray(v) for k, v in inputs.items()}
    obs = inp["obs"].astype(np.float32)
    scale = np.asarray(inp["scale"], np.float32)
    mean = np.asarray(inp["mean"], np.float32)

    key = (float(scale[0]), float(scale[1]), float(mean[0]), float(mean[1]))
    if key not in _BUILD_CACHE:
        _BUILD_CACHE[key] = build(*key)
    nc = _BUILD_CACHE[key]

    wpack, wfix, biaspack = _pack_weights(inp)
    in_maps = []
    for c in range(NCORES):
        invT, locvel = _arrange_inputs(obs[c * NODES_CORE:(c + 1) * NODES_CORE])
        in_maps.append({"invT": invT, "locvel": locvel, "wpack": wpack,
                        "wfix": wfix, "biaspack": biaspack})
    res = run_bass_kernel_spmd(nc, in_maps, list(range(NCORES)))
    outs = [_unarrange_output(res.results[c]["out"]) for c in range(NCORES)]
    return np.concatenate(outs, axis=0)


# revision 17
# speedup vs baseline: 1.6052x; 1.3829x over previous
"""Trainium2 Bass kernel for nn_DeterministicEgnnPolicy (EGNN message passing).

Strategy (per sharding hint): shard the 1024 independent 32-node graphs
across 8 NeuronCores (128 graphs/core). On each core the fully-connected
edge structure is computed densely as all-pairs 32x32 blocks:

- "feature-major" edge tensors [128 = 2 graph-halves x 64 features,
  (i, j)] drive the edge-MLP matmuls with block-diagonal weights
  (fp16, full 128-partition contraction, fp32 PSUM accumulate).
- per-edge scalars (radial, 1/(1+sqrt(r)), u = t*s, aggregations) live in a
  "matrix" layout [128 = (gm, i), 32x32 = (gb, j)], 64x cheaper for
  DVE/ACT; batched SBUF->SBUF DMAs convert between the layouts.
- the edge phase is software-pipelined across units so the PE / ACT / DVE
  queues interleave stages of different units (no head-of-line blocking).

Graph indexing on a core: g = gb*4 + gm, gb in [0,32), gm in [0,4).
half = gb//16 (feature partitions 64*half..64*half+63).
node free index (per half): n' = gb_l*128 + gm*32 + i, gb_l = gb%16.
global node: n = gb*128 + gm*32 + i.
"""

import numpy as np

N_AGENTS = 32
BATCH = 1024
H = 64
L = 4
INV = 16
DEG = float(N_AGENTS - 1)
NCORES = 8
G_CORE = BATCH // NCORES          # 128 graphs per core
NGB = G_CORE // 4                 # 32 gb blocks per core
NGBL = NGB // 2                   # 16 per half
NNODE = NGBL * 128                # 2048 node free dim (per half)
NODES_CORE = G_CORE * N_AGENTS    # 4096
NUNIT = 64                        # units per layer (one graph-pair each)

_BUILD_CACHE = {}


# ----------------------------------------------------------------------------
# Host-side packing (pure layout permutation / weight arrangement)
# ----------------------------------------------------------------------------

def _bd(w):
    """64x64 block-diagonal lhsT [128,128] from w [64,64] (or [k,64])."""
    k = w.shape[0]
    out = np.zeros((128, 128), np.float32)
    out[0:k, 0:64] = w
    out[64:64 + k, 64:128] = w
    return out


def _bd_rep(wcol):
    """Replicating lhsT: out[64h+f, 64h+f'] = wcol[f] for all f'."""
    out = np.zeros((128, 128), np.float32)
    col = wcol.reshape(64, 1)
    out[0:64, 0:64] = np.repeat(col, 64, axis=1)
    out[64:128, 64:128] = np.repeat(col, 64, axis=1)
    return out


def _bd4(w):
    """All-quadrant lhsT [128,128]: w [64,64] in each 64x64 quadrant."""
    out = np.zeros((128, 128), np.float32)
    for a in (0, 64):
        for b in (0, 64):
            out[a:a + 64, b:b + 64] = w
    return out


def _bd_rep4(wcol):
    """Replicating lhsT: out[64a+f, c] = wcol[f] for all c."""
    return np.repeat(np.tile(wcol.reshape(64, 1), (2, 1)), 128, axis=1)


def _pack_weights(inp):
    """Build wpack [128, NW*128] fp16, wfix [128, 256] f32, biaspack f32."""
    tiles = []
    names = []

    def add(name, arr):
        t = np.zeros((128, 128), np.float32)
        t[:arr.shape[0], :arr.shape[1]] = arr
        tiles.append(t)
        names.append(name)

    emb = np.zeros((128, 128), np.float32)
    emb[0:INV, 0:64] = inp["emb_W"]
    emb[64:64 + INV, 64:128] = inp["emb_W"]
    add("emb", emb)

    for l in range(L):
        We1 = inp["We1"][l]          # [130, 64]
        add(f"Wi{l}", _bd(We1[0:64]))
        add(f"Wj{l}", _bd(We1[64:128]))
        wsc = np.zeros((4, 128), np.float32)
        wsc[0, 0:64] = We1[128]      # radial, half0
        wsc[1, 0:64] = We1[129]      # edge_attr, half0
        wsc[2, 64:128] = We1[128]
        wsc[3, 64:128] = We1[129]
        add(f"Wsc{l}", wsc)
        add(f"We2{l}", _bd(inp["We2"][l]))
        add(f"Wc1{l}", _bd(inp["Wc1"][l]))
        add(f"Wc2{l}", _bd_rep(inp["Wc2"][l][:, 0]))
        add(f"Wv1{l}", _bd(inp["Wv1"][l]))
        add(f"Wv2{l}", _bd_rep(inp["Wv2"][l][:, 0]))
        Wn1 = inp["Wn1"][l]          # [128, 64]
        add(f"Wn1t{l}", _bd(Wn1[0:64]))
        add(f"Wn1b{l}", _bd(Wn1[64:128]))
        add(f"Wn2{l}", _bd(inp["Wn2"][l]))

    wpack = np.concatenate(tiles, axis=1).astype(np.float16)

    ident = np.eye(128, dtype=np.float32)
    delta = np.zeros((128, 128), np.float32)
    for gm in range(4):
        delta[gm, gm * 32:(gm + 1) * 32] = 1.0
    wfix = np.concatenate([ident, delta], axis=1).astype(np.float32)

    bias_cols = []
    bnames = []
    for l in range(L):
        for nm in ("be1", "be2", "bc1", "bv1", "bn1", "bn2"):
            bias_cols.append(np.tile(inp[nm][l].reshape(-1), 2))
            bnames.append(f"{nm}{l}")
        for nm in ("bv2", "bc2"):
            bias_cols.append(np.full(128, float(inp[nm][l].reshape(-1)[0]), np.float32))
            bnames.append(f"{nm}{l}")
    bias_cols.append(np.tile(inp["emb_b"], 2))
    bnames.append("embb")
    biaspack = np.stack(bias_cols, axis=1).astype(np.float32)  # [128, NB]
    return wpack, wfix, biaspack


def _arrange_inputs(obs_slice):
    """Per-core obs slice [4096, 20] -> invT fp16 [128, 2048], locvel f32."""
    obs3 = obs_slice.reshape(NGB, 128, 20)          # [gb, (gm,i), col]
    invT = np.zeros((128, NNODE), np.float32)
    inv_half0 = obs3[0:NGBL, :, 0:INV]              # [16, 128, 16]
    inv_half1 = obs3[NGBL:NGB, :, 0:INV]
    invT[0:INV, :] = np.transpose(inv_half0, (2, 0, 1)).reshape(INV, NNODE)
    invT[64:64 + INV, :] = np.transpose(inv_half1, (2, 0, 1)).reshape(INV, NNODE)
    locvel = np.ascontiguousarray(
        np.transpose(obs3[:, :, INV:INV + 4], (1, 0, 2)).reshape(128, NGB * 4)
    ).astype(np.float32)
    return invT.astype(np.float16), locvel


def _unarrange_output(outP):
    """outP [128, 64] -> [4096, 2] (n = gb*128 + p)."""
    return np.ascontiguousarray(
        outP.reshape(128, NGB, 2).transpose(1, 0, 2).reshape(NODES_CORE, 2)
    )


# ----------------------------------------------------------------------------
# Device kernel builder
# ----------------------------------------------------------------------------

def build(scale0, scale1, mean0, mean1):
    import concourse.bacc as bacc
    import concourse.tile as tile
    import concourse.mybir as mybir
    from contextlib import ExitStack

    F32 = mybir.dt.float32
    F16 = mybir.dt.float16
    AT = mybir.AluOpType
    ACTF = mybir.ActivationFunctionType

    nc = bacc.Bacc("TRN2", target_bir_lowering=False, debug=False)

    invT_d = nc.dram_tensor("invT", [128, NNODE], F16, kind="ExternalInput")
    locvel_d = nc.dram_tensor("locvel", [128, NGB * 4], F32, kind="ExternalInput")
    NW = 1 + 11 * L
    wpack_d = nc.dram_tensor("wpack", [128, NW * 128], F16, kind="ExternalInput")
    wfix_d = nc.dram_tensor("wfix", [128, 256], F32, kind="ExternalInput")
    NBIAS = 8 * L + 1
    bias_d = nc.dram_tensor("biaspack", [128, NBIAS], F32, kind="ExternalInput")
    out_d = nc.dram_tensor("out", [128, NGB * 2], F32, kind="ExternalOutput")

    # weight tile indices (must match _pack_weights order)
    widx = {}
    _wi = 0
    for nm in ("emb",):
        widx[nm] = _wi
        _wi += 1
    for l in range(L):
        for nm in ("Wi", "Wj", "Wsc", "We2", "Wc1", "Wc2", "Wv1", "Wv2",
                   "Wn1t", "Wn1b", "Wn2"):
            widx[f"{nm}{l}"] = _wi
            _wi += 1
    assert _wi == NW
    bidx = {}
    _bi = 0
    for l in range(L):
        for nm in ("be1", "be2", "bc1", "bv1", "bn1", "bn2", "bv2", "bc2"):
            bidx[f"{nm}{l}"] = _bi
            _bi += 1
    bidx["embb"] = _bi

    with tile.TileContext(nc) as tc, ExitStack() as ctx:
        st = ctx.enter_context(tc.tile_pool(name="static", bufs=1))
        eA = ctx.enter_context(tc.tile_pool(name="eA", bufs=3))
        eM = ctx.enter_context(tc.tile_pool(name="eM", bufs=3))
        eC = ctx.enter_context(tc.tile_pool(name="eC", bufs=3))
        eS = ctx.enter_context(tc.tile_pool(name="eS", bufs=3))
        eR = ctx.enter_context(tc.tile_pool(name="eR", bufs=2))
        mx = ctx.enter_context(tc.tile_pool(name="mx", bufs=1))
        # PSUM: 8 banks: ps1 2x[128,1024] + ps2 1x + ps34 1x
        ps1p = ctx.enter_context(tc.tile_pool(name="ps1p", bufs=2, space="PSUM"))
        ps2p = ctx.enter_context(tc.tile_pool(name="ps2p", bufs=1, space="PSUM"))
        ps34 = ctx.enter_context(tc.tile_pool(name="ps34", bufs=1, space="PSUM"))

        # ---- static loads ----
        wsb = st.tile([128, NW * 128], F16)
        nc.sync.dma_start(wsb[:], wpack_d.ap())
        wfx = st.tile([128, 256], F32)
        nc.sync.dma_start(wfx[:], wfix_d.ap())
        bsb = st.tile([128, NBIAS], F32)
        nc.sync.dma_start(bsb[:], bias_d.ap())
        invT = st.tile([128, NNODE], F16)
        nc.sync.dma_start(invT[:], invT_d.ap())
        locvel = st.tile([128, NGB * 4], F32)
        nc.sync.dma_start(locvel[:], locvel_d.ap())

        def W(name):
            return wsb[:, widx[name] * 128:(widx[name] + 1) * 128]

        def Bia(name):
            return bsb[:, bidx[name]:bidx[name] + 1]

        ident = wfx[:, 0:128]
        delta4 = wfx[0:4, 128:256]

        # ---- persistent state ----
        hA = st.tile([128, NNODE], F16)
        hB = st.tile([128, NNODE], F16)
        magg = st.tile([128, NNODE], F16)
        smat = st.tile([128, 1024], F32)
        rad = st.tile([128, 1024], F32)
        radf = st.tile([128, 1024], F16)
        eaf = st.tile([128, 1024], F16)
        dx = st.tile([128, 1024], F32)
        dy = st.tile([128, 1024], F32)
        locx = st.tile([128, NGB], F32)
        locy = st.tile([128, NGB], F32)
        velx = st.tile([128, NGB], F32)
        vely = st.tile([128, NGB], F32)
        phiA = st.tile([128, NGB], F32)
        phiB = st.tile([128, NGB], F32)
        hv1 = st.tile([128, NNODE], F16)
        phirep = st.tile([128, NNODE], F32)
        lxT = st.tile([32, 128], F32)
        lyT = st.tile([32, 128], F32)
        T4x = st.tile([4, 1024], F32)
        T4y = st.tile([4, 1024], F32)
        outP = st.tile([128, NGB * 2], F32)

        lv = locvel[:].rearrange("p (gb c) -> p gb c", c=4)
        nc.vector.tensor_copy(locx[:], lv[:, :, 0])
        nc.vector.tensor_copy(locy[:], lv[:, :, 1])
        nc.vector.tensor_copy(velx[:], lv[:, :, 2])
        nc.vector.tensor_copy(vely[:], lv[:, :, 3])

        def heat(lhsT_ap, rhs_ap, n=12):
            hp = ps34.tile([128, 512], F32, tag="s34")
            for _ in range(n):
                nc.tensor.matmul(hp[:], lhsT_ap, rhs_ap, start=True, stop=True)

        def radial_part(first):
            """Compute lxT/lyT, T4s, dx, dy, rad (+fp16 copies) from locx/locy."""
            for (lP, lT) in ((locx, lxT), (locy, lyT)):
                pst = ps34.tile([32, 128], F32, tag="s34")
                nc.tensor.transpose(pst[:], lP[:], ident)
                nc.vector.tensor_copy(lT[:], pst[:])
            for (lT, T4) in ((lxT, T4x), (lyT, T4y)):
                for gm in range(4):
                    nc.sync.dma_start(
                        T4[gm:gm + 1, :].rearrange("p (gb j) -> p gb j", j=32),
                        lT[:, gm * 32:(gm + 1) * 32])
            for (T4, lP, dT) in ((T4x, locx, dx), (T4y, locy, dy)):
                pss = ps2p.tile([128, 1024], F32, tag="s2")
                for k in range(2):
                    nc.tensor.matmul(pss[:, k * 512:(k + 1) * 512], delta4,
                                     T4[:, k * 512:(k + 1) * 512],
                                     start=True, stop=True)
                bc = lP[:].unsqueeze(2).broadcast_to([128, NGB, 32])
                nc.vector.tensor_tensor(
                    dT[:].rearrange("p (gb j) -> p gb j", j=32), bc,
                    pss[:].rearrange("p (gb j) -> p gb j", j=32), op=AT.subtract)
            t2 = mx.tile([128, 1024], F32, tag="mx_t2")
            nc.vector.tensor_tensor(rad[:], dx[:], dx[:], op=AT.mult)
            nc.vector.tensor_tensor(t2[:], dy[:], dy[:], op=AT.mult)
            nc.vector.tensor_tensor(rad[:], rad[:], t2[:], op=AT.add)
            with nc.allow_low_precision(reason="fp16 radial for edge MLP"):
                nc.vector.tensor_copy(radf[:], rad[:])
                if first:
                    nc.vector.tensor_copy(eaf[:], radf[:])

        radial_part(first=True)

        # rsc group DMAs: group g holds chunks (gb_l in {2g,2g+1}) x gmp,
        # free layout (gmp, u, i, gl, j) row-major per row.
        def rsc_dma(rsc, g):
            for hh in range(2):
                for (r, src) in ((2 * hh, radf), (2 * hh + 1, eaf)):
                    nc.sync.dma_start(
                        rsc[r:r + 1, :].rearrange(
                            "p (gmp u i gl j) -> p gmp u i gl j",
                            gmp=2, u=2, i=32, gl=2, j=32),
                        src[:, (2 * g + 16 * hh) * 32:(2 * g + 2 + 16 * hh) * 32]
                            .rearrange("p (gl j) -> p gl j", j=32))

        # ---- embedding: h0 = inv @ emb_W + emb_b ----
        heat(W("emb"), invT[:, 0:512])
        for u in range(NNODE // 1024):
            pse = ps1p.tile([128, 1024], F32, tag="s1")
            for k in range(2):
                nc.tensor.matmul(pse[:, k * 512:(k + 1) * 512], W("emb"),
                                 invT[:, u * 1024 + k * 512:u * 1024 + (k + 1) * 512],
                                 start=True, stop=True)
            with nc.allow_low_precision(reason="fp16 h"):
                nc.vector.tensor_scalar_add(hA[:, u * 1024:(u + 1) * 1024],
                                            pse[:], Bia("embb"))

        def node_phase(l, h, phiP):
            """phi_v for layer l from h -> phiP (matrix layout)."""
            for u in range(NNODE // 1024):
                sl = slice(u * 1024, (u + 1) * 1024)
                psv = ps2p.tile([128, 1024], F32, tag="s2")
                for k in range(2):
                    ksl = slice(u * 1024 + k * 512, u * 1024 + (k + 1) * 512)
                    nc.tensor.matmul(psv[:, k * 512:(k + 1) * 512],
                                     W(f"Wv1{l}"), h[:, ksl],
                                     start=True, stop=True)
                with nc.allow_low_precision(reason="fp16 hv1"):
                    nc.scalar.activation(hv1[:, sl], psv[:], ACTF.Silu,
                                         bias=Bia(f"bv1{l}"))
                psv2 = ps2p.tile([128, 1024], F32, tag="s2")
                for k in range(2):
                    nc.tensor.matmul(psv2[:, k * 512:(k + 1) * 512],
                                     W(f"Wv2{l}"),
                                     hv1[:, u * 1024 + k * 512:u * 1024 + (k + 1) * 512],
                                     start=True, stop=True)
                nc.vector.tensor_scalar_add(phirep[:, sl], psv2[:], Bia(f"bv2{l}"))
            # phiP[(gm,i), gb=hh*16+gb_l] = phirep[64*hh, gb_l*128+gm*32+i]
            for c in range(NGBL):
                pst = ps34.tile([128, 128], F32, tag="s34")
                nc.tensor.transpose(pst[:], phirep[:, c * 128:(c + 1) * 128], ident)
                nc.vector.tensor_copy(phiP[:, c:c + 1], pst[:, 0:1])
                nc.vector.tensor_copy(phiP[:, c + NGBL:c + NGBL + 1], pst[:, 64:65])

        node_phase(0, hA, phiA)

        for l in range(L):
            h = hA if l % 2 == 0 else hB
            h_next = hB if l % 2 == 0 else hA
            phiP = phiA if l % 2 == 0 else phiB
            phiN = phiB if l % 2 == 0 else phiA

            # ---- edge phase: software-pipelined over 64 units ----
            # unit t: gb_l = t//4, gmp = (t//2)%2, u = t%2, gl = (t//4)%2
            rsc_tiles = {}
            rsc_tiles[0] = eR.tile([4, 8192], F16, tag="rsc", name="rsc0")
            rsc_dma(rsc_tiles[0], 0)
            st_m1, st_mu, st_c1, st_s4 = {}, {}, {}, {}

            def stage1(t):
                gb_l, gmp, u, gl = t // 4, (t // 2) % 2, t % 2, (t // 4) % 2
                g = t // 8
                if t % 8 == 0 and g + 1 < 8:
                    rsc_tiles[g + 1] = eR.tile([4, 8192], F16, tag="rsc", name=f"rsc{g+1}")
                    rsc_dma(rsc_tiles[g + 1], g + 1)
                nb = gb_l * 128 + gmp * 64 + u * 32
                rv = rsc_tiles[g][0:4, :].rearrange(
                    "p (gmp u i gl j) -> p gmp u i gl j",
                    gmp=2, u=2, i=32, gl=2, j=32)
                ps1 = ps1p.tile([128, 1024], F32, tag="s1")
                for k in range(2):
                    ksl = slice(k * 512, (k + 1) * 512)
                    hi = h[:, nb + k * 16:nb + (k + 1) * 16]
                    hi_bc = hi.unsqueeze(2).broadcast_to([128, 16, 32])
                    hj = h[:, nb:nb + 32]
                    hj_bc = hj.unsqueeze(1).broadcast_to([128, 16, 32])
                    nc.tensor.matmul(ps1[:, ksl], W(f"Wi{l}"),
                                     hi_bc, start=True, stop=False)
                    nc.tensor.matmul(ps1[:, ksl], W(f"Wj{l}"),
                                     hj_bc, start=False, stop=False)
                    nc.tensor.matmul(ps1[:, ksl], W(f"Wsc{l}")[0:4, :],
                                     rv[:, gmp, u, 16 * k:16 * (k + 1), gl, :],
                                     start=False, stop=True)
                m1s = eA.tile([128, 1024], F16, tag="m1s")
                with nc.allow_low_precision(reason="fp16 edge mlp"):
                    nc.scalar.activation(m1s[:], ps1[:], ACTF.Silu,
                                         bias=Bia(f"be1{l}"))
                st_m1[t] = m1s

            def stage2(t):
                nb = (t // 4) * 128 + ((t // 2) % 2) * 64 + (t % 2) * 32
                m1s = st_m1.pop(t)
                ps2 = ps2p.tile([128, 1024], F32, tag="s2")
                for k in range(2):
                    ksl = slice(k * 512, (k + 1) * 512)
                    nc.tensor.matmul(ps2[:, ksl], W(f"We2{l}"),
                                     m1s[:, ksl], start=True, stop=True)
                m_u = eM.tile([128, 1024], F16, tag="m_u")
                with nc.allow_low_precision(reason="fp16 edge mlp"):
                    nc.scalar.activation(m_u[:], ps2[:], ACTF.Silu,
                                         bias=Bia(f"be2{l}"))
                    nc.vector.memset(m_u[:, 0:1024:33], 0.0)
                    nc.vector.tensor_reduce(
                        magg[:, nb:nb + 32],
                        m_u[:].rearrange("p (i j) -> p i j", j=32),
                        axis=mybir.AxisListType.X, op=AT.add)
                st_mu[t] = m_u

            def stage3(t):
                m_u = st_mu.pop(t)
                ps3 = ps34.tile([128, 1024], F32, tag="s34")
                for k in range(2):
                    ksl = slice(k * 512, (k + 1) * 512)
                    nc.tensor.matmul(ps3[:, ksl], W(f"Wc1{l}"),
                                     m_u[:, ksl], start=True, stop=True)
                c1 = eC.tile([128, 1024], F16, tag="c1")
                with nc.allow_low_precision(reason="fp16 edge mlp"):
                    nc.scalar.activation(c1[:], ps3[:], ACTF.Silu,
                                         bias=Bia(f"bc1{l}"))
                st_c1[t] = c1

            def stage4(t):
                gb_l, gmp, u = t // 4, (t // 2) % 2, t % 2
                c1 = st_c1.pop(t)
                ps4 = ps34.tile([128, 1024], F32, tag="s34")
                for k in range(2):
                    ksl = slice(k * 512, (k + 1) * 512)
                    nc.tensor.matmul(ps4[:, ksl], W(f"Wc2{l}"),
                                     c1[:, ksl], start=True, stop=True)
                ssb = eS.tile([128, 1024], F32, tag="ssb")
                nc.vector.tensor_scalar_add(ssb[:], ps4[:], Bia(f"bc2{l}"))
                pg = (gmp * 2 + u) * 32
                nc.sync.dma_start(
                    smat[pg:pg + 32, gb_l * 32:(gb_l + 1) * 32]
                        .rearrange("p (hb j) -> p hb j", hb=1),
                    ssb[0:1, :].rearrange("p (i j) -> p i j", j=32))
                nc.sync.dma_start(
                    smat[pg:pg + 32, (gb_l + 16) * 32:(gb_l + 17) * 32]
                        .rearrange("p (hb j) -> p hb j", hb=1),
                    ssb[64:65, :].rearrange("p (i j) -> p i j", j=32))

            # ---- h update (no diagonal correction: m_u diag zeroed) ----
            def h_update(u):
                sl = slice(u * 1024, (u + 1) * 1024)
                psh = ps1p.tile([128, 1024], F32, tag="s1")
                for k in range(2):
                    ksl = slice(u * 1024 + k * 512, u * 1024 + (k + 1) * 512)
                    osl = slice(k * 512, (k + 1) * 512)
                    nc.tensor.matmul(psh[:, osl], W(f"Wn1t{l}"),
                                     h[:, ksl], start=True, stop=False)
                    nc.tensor.matmul(psh[:, osl], W(f"Wn1b{l}"),
                                     magg[:, ksl], start=False, stop=True)
                hn1 = eA.tile([128, 1024], F16, tag="hn1")
                with nc.allow_low_precision(reason="fp16 edge mlp"):
                    nc.scalar.activation(hn1[:], psh[:], ACTF.Silu,
                                         bias=Bia(f"bn1{l}"))
                psh2 = ps2p.tile([128, 1024], F32, tag="s2")
                for k in range(2):
                    osl = slice(k * 512, (k + 1) * 512)
                    nc.tensor.matmul(psh2[:, osl], W(f"Wn2{l}"),
                                     hn1[:, osl], start=True, stop=True)
                with nc.allow_low_precision(reason="fp16 h"):
                    nc.vector.scalar_tensor_tensor(
                        h_next[:, sl], psh2[:], Bia(f"bn2{l}"), h[:, sl],
                        op0=AT.add, op1=AT.add)


            for t in range(NUNIT + 3):
                if t < NUNIT:
                    stage1(t)
                if 1 <= t < NUNIT + 1:
                    stage2(t - 1)
                if 2 <= t < NUNIT + 2:
                    stage3(t - 2)
                if 3 <= t:
                    stage4(t - 3)
                if t == 3:
                    sq = mx.tile([128, 1024], F32, tag="mx_sq")
                    nc.scalar.activation(sq[:], rad[:], ACTF.Sqrt)
                    nc.vector.tensor_scalar_add(sq[:], sq[:], 1.0)
                    tm = mx.tile([128, 1024], F32, tag="mx_tm")
                    nc.vector.reciprocal_approx_fast(tm[:], sq[:])


            h_update(0)
            h_update(1)

            # next layer's node/phi phase first: overlaps the mx phase below
            if l < L - 1:
                node_phase(l + 1, h_next, phiN)

            # ---- matrix phase: t, u, agg, vel/loc update; then radial(l+1) ----
            um = mx.tile([128, 1024], F32, tag="mx_um")
            nc.vector.tensor_tensor(um[:], tm[:], smat[:], op=AT.mult)
            for (dT, agg_out) in ((dx, "ax"), (dy, "ay")):
                w_ = mx.tile([128, 1024], F32, tag="mx_w")
                nc.vector.tensor_tensor(w_[:], um[:], dT[:], op=AT.mult)
                ag = mx.tile([128, NGB], F32, tag="mx_" + agg_out)
                nc.vector.tensor_reduce(
                    ag[:], w_[:].rearrange("p (gb j) -> p gb j", j=32),
                    axis=mybir.AxisListType.X, op=AT.add)
                vP = velx if agg_out == "ax" else vely
                tmp = mx.tile([128, NGB], F32, tag="mx_tmp")
                nc.vector.tensor_tensor(tmp[:], phiP[:], vP[:], op=AT.mult)
                nc.vector.scalar_tensor_tensor(vP[:], ag[:], 1.0 / DEG, tmp[:],
                                               op0=AT.mult, op1=AT.add)
            nc.vector.tensor_tensor(locx[:], locx[:], velx[:], op=AT.add)
            nc.vector.tensor_tensor(locy[:], locy[:], vely[:], op=AT.add)
            if l < L - 1:
                radial_part(first=False)

        # ---- output: outP interleaved (gb, c) ----
        ov = outP[:].rearrange("p (gb c) -> p gb c", c=2)
        nc.vector.tensor_scalar(ov[:, :, 0], velx[:], scale0, mean0,
                                op0=AT.mult, op1=AT.add)
        nc.vector.tensor_scalar(ov[:, :, 1], vely[:], scale1, mean1,
                                op0=AT.mult, op1=AT.add)
        nc.sync.dma_start(out_d.ap(), outP[:])

    nc.compile()
    return nc


# ----------------------------------------------------------------------------
# Entry point
# ----------------------------------------------------------------------------

def kernel(**inputs):
    import concourse.mybir  # noqa: F401  (ensure env importable)
    from concourse.bass_utils import run_bass_kernel_spmd

    inp = {k: np.asarray(v) for k, v in inputs.items()}
    obs = inp["obs"].astype(np.float32)
    scale = np.asarray(inp["scale"], np.float32)
    mean = np.asarray(inp["mean"], np.float32)

    key = (float(scale[0]), float(scale[1]), float(mean[0]), float(mean[1]))
    if key not in _BUILD_CACHE:
        _BUILD_CACHE[key] = build(*key)
    nc = _BUILD_CACHE[key]

    wpack, wfix, biaspack = _pack_weights(inp)
    in_maps = []
    for c in range(NCORES):
        invT, locvel = _arrange_inputs(obs[c * NODES_CORE:(c + 1) * NODES_CORE])
        in_maps.append({"invT": invT, "locvel": locvel, "wpack": wpack,
                        "wfix": wfix, "biaspack": biaspack})
    res = run_bass_kernel_spmd(nc, in_maps, list(range(NCORES)))
    outs = [_unarrange_output(res.results[c]["out"]) for c in range(NCORES)]
    return np.concatenate(outs, axis=0)
